# revision 29
# baseline (speedup 1.0000x reference)
"""Biaffine NER model (2-layer BiLSTM + highway + biaffine) on 8 Trainium2 cores.

Strategy:
  - Data-parallel over batch: each of the 8 cores handles B_loc=2 of the 16
    batch elements, full model, no collectives.
  - The LSTM recurrences are solved by fixed-point (Jacobi) iteration:
      H^{k+1} = LSTMCell(x_tilde + shift(H^k) @ W_h)
    Each iteration is fully parallel over time (big matmuls, M = B_loc*T = 512
    rows), and the cell-state recurrence c_t = a_t*c_{t-1} + b_t is computed
    with the hardware tensor_tensor_scan. The map contracts by ~4x per
    iteration (validated vs the reference); K_ITERS=8 reaches the fp32r
    rounding floor (~6e-3 rel absmax).
  - Everything on-chip is kept "transposed" (feature-major, [128-partition
    folds, (b, t) free]) so matmuls, activations and scans all operate on
    full-width tiles.
  - Matmuls run as fp32r (1 cycle/row at moving dim >= 256).
  - Biases ride as an extra contraction row (ones row in the moving operand,
    bias row in the stationary operand).
"""

import sys

sys.path.insert(0, "/opt/trn_rl_repo")

import ml_dtypes
import numpy as np

import concourse.bass as bass
import concourse.mybir as mybir
import concourse.tile as tile
from concourse.bass_utils import run_bass_kernel_spmd
from concourse.masks import make_identity

F32 = mybir.dt.float32
BF16 = mybir.dt.bfloat16
BF16NP = ml_dtypes.bfloat16
AF = mybir.ActivationFunctionType

B, T, D = 16, 256, 768
H, H2, G = 400, 800, 1200
F, C = 150, 8
NCORES = 8
BL = B // NCORES          # 2 batch elements per core
L = BL * T                # 512 (b, t) rows per core
GP = 512                  # per-gate padded stride (3*GP = 1536, 12 M-tiles)
NM = 12                   # M-tiles of the padded gate dim
KH = [(0, 128), (128, 256), (256, 384), (384, 401)]   # K-tiles of [H+1] (bias row at 400)
KD = [(k * 128, (k + 1) * 128) for k in range(6)]      # K-tiles of D=768
K_ITERS = 6

_CACHE = {}


def _r(ap):
    return ap


# ------------------------------------------------------------------ host packing

def _pack_gate_cols(w):
    """[K, 3H] -> [K, 3*GP] with each gate's 400 cols padded to 512."""
    k = w.shape[0]
    out = np.zeros((k, 3 * GP), np.float32)
    for g in range(3):
        out[:, g * GP:g * GP + H] = w[:, g * H:(g + 1) * H]
    return np.ascontiguousarray(out)


def _with_bias_row(w, bias):
    """Append one row (the bias, already packed like w's columns) to w."""
    return np.ascontiguousarray(np.concatenate([w, bias[None, :]], 0))


def _fold128(v, nchunk):
    """[n] -> [128, nchunk] column-major fold (unit u -> [u%128, u//128])."""
    out = np.zeros((128, nchunk), np.float32)
    n = len(v)
    for m in range(nchunk):
        seg = v[m * 128:min((m + 1) * 128, n)]
        out[:len(seg), m] = seg
    return out


def _pack_inputs(inputs):
    """Pack weights into the DRAM layouts the program expects (shared by all cores)."""
    f32 = lambda a: np.ascontiguousarray(np.asarray(a, np.float32))
    x = f32(inputs["x"])
    z = np.zeros((3 * GP,), np.float32)

    packs = {}
    # layer 0: W [D+H, 3H].  The gate bias rides as the last row of wh (it is
    # re-added every Jacobi iteration through the ones slot of ht).
    for nm, wn, bn in (("0f", "W_f0", "b_f0"), ("0b", "W_b0", "b_b0")):
        W = f32(inputs[wn]); bias = _pack_gate_cols(f32(inputs[bn])[None, :])[0]
        packs["wx" + nm] = _pack_gate_cols(W[:D])
        packs["wh" + nm] = _with_bias_row(_pack_gate_cols(W[D:]), bias)
    # layer 1: W [2H+H, 3H]; the input half splits into hf/hb parts (both with
    # zero bias rows — the bias lives only in wh).
    for nm, wn, bn in (("1f", "W_f1", "b_f1"), ("1b", "W_b1", "b_b1")):
        W = f32(inputs[wn]); bias = _pack_gate_cols(f32(inputs[bn])[None, :])[0]
        packs["wx" + nm + "f"] = _with_bias_row(_pack_gate_cols(W[:H]), z)
        packs["wx" + nm + "b"] = _with_bias_row(_pack_gate_cols(W[H:H2]), z)
        packs["wh" + nm] = _with_bias_row(_pack_gate_cols(W[H2:]), bias)

    # highway: W_hw [2H, 2H]; M packed as [f-half pad 512 | b-half pad 512]
    Whw = f32(inputs["W_hw"]); bhw = f32(inputs["b_hw"])

    def _pack_hw_cols(w):
        k = w.shape[0]
        out = np.zeros((k, 2 * GP), np.float32)
        out[:, 0:H] = w[:, 0:H]
        out[:, GP:GP + H] = w[:, H:H2]
        return out

    zh = np.zeros((2 * GP,), np.float32)
    packs["whwf"] = _with_bias_row(_pack_hw_cols(Whw[:H]), _pack_hw_cols(bhw[None, :])[0])
    packs["whwb"] = _with_bias_row(_pack_hw_cols(Whw[H:]), zh)

    # projections: Ws/We [2H, F]
    for nm, wn, bn in (("s", "W_s", "b_s"), ("e", "W_e", "b_e")):
        W = f32(inputs[wn]); bias = f32(inputs[bn])
        packs["w" + nm + "f"] = _with_bias_row(W[:H], bias)
        packs["w" + nm + "b"] = _with_bias_row(W[H:], np.zeros((F,), np.float32))

    # biaffine U [F+1, C, F+1] -> [F+1, C*256] (each c padded 151->256)
    U = f32(inputs["U"])
    upk = np.zeros((F + 1, C * 256), np.float32)
    for c in range(C):
        upk[:, c * 256:c * 256 + F + 1] = U[:, c, :]
    packs["upk"] = np.ascontiguousarray(upk)

    packs = {k: v.astype(BF16NP) for k, v in packs.items()}
    h0f = _fold128(f32(inputs["h0"])[0], 4)
    hti = np.zeros((128, 4, BL, T + 1), np.float32)
    hti[:, :, :, 0] = h0f[:, :, None]          # slot 0 = h0
    hti[16, 3, :, :] = 1.0                     # ones rail for the bias rows
    packs["hti"] = hti.astype(BF16NP)
    packs["c0f"] = _fold128(f32(inputs["c0"])[0], 4)

    # per-core x, feature-major [D, L], normal and time-reversed
    xr = x[:, ::-1]
    per_core = []
    for c in range(NCORES):
        sl = x[c * BL:(c + 1) * BL]
        slr = xr[c * BL:(c + 1) * BL]
        m = dict(packs)
        m["xT"] = np.ascontiguousarray(sl.transpose(2, 0, 1).reshape(D, L)).astype(BF16NP)
        m["xTr"] = np.ascontiguousarray(slr.transpose(2, 0, 1).reshape(D, L)).astype(BF16NP)
        per_core.append(m)
    return per_core


# ------------------------------------------------------------------ program

def _build_program():
    nc = bass.Bass(trn_type="TRN2", target_bir_lowering=False, debug=False)

    dins = {}

    def din(name, shape, dt=BF16):
        dins[name] = nc.dram_tensor(name, list(shape), dt, kind="ExternalInput").ap()
        return dins[name]

    din("xT", (D, L)); din("xTr", (D, L))
    din("wx0f", (D, 3 * GP)); din("wx0b", (D, 3 * GP))
    din("wh0f", (H + 1, 3 * GP)); din("wh0b", (H + 1, 3 * GP))
    for s in ("1f", "1b"):
        din("wx" + s + "f", (H + 1, 3 * GP))
        din("wx" + s + "b", (H + 1, 3 * GP))
        din("wh" + s, (H + 1, 3 * GP))
    din("whwf", (H + 1, 2 * GP)); din("whwb", (H + 1, 2 * GP))
    din("wsf", (H + 1, F)); din("wsb", (H + 1, F))
    din("wef", (H + 1, F)); din("web", (H + 1, F))
    din("upk", (F + 1, C * 256))
    din("hti", (128, 4, BL, T + 1)); din("c0f", (128, 4), dt=F32)
    out_d = nc.dram_tensor("out", [BL, T, T, C], F32, kind="ExternalOutput").ap()

    with tile.TileContext(nc) as tc:
        _body(nc, tc, dins, out_d)
    _split_multi_waits(nc)
    return nc


def _split_multi_waits(nc, max_waits=1):
    """This container's walrus supports only one embedded sync-wait per
    instruction ("Too many sync wait commands"); hoist extra waits onto
    single-wait NoOps inserted just before, on the same engine queue.
    Sequential waiting on monotone semaphores is equivalent to the joint
    wait."""
    n = 0
    for func in nc.m.functions:
        for blk in func.blocks:
            out = []
            for inst in blk.instructions:
                si = inst.sync_info
                if si is not None and si.on_wait and len(si.on_wait) > max_waits:
                    waits = list(si.on_wait)
                    for j, w in enumerate(waits[:-max_waits]):
                        nop = mybir.InstNoOp(name=f"{inst.name}-xw{j}")
                        nop.engine = inst.engine
                        nop.sync_info = mybir.SyncInfo(on_wait=[w], on_update=[])
                        out.append(nop)
                        n += 1
                    inst.sync_info = mybir.SyncInfo(
                        on_wait=waits[-max_waits:], on_update=list(si.on_update))
                out.append(inst)
            blk.instructions = out
    return n


def _load_ktiles(nc, pool, dram, ktiles, cols, tagp):
    tiles = []
    for i, (a, b) in enumerate(ktiles):
        t = pool.tile([b - a, cols], BF16, name=f"{tagp}_{i}", tag=f"{tagp}_{i}")
        nc.sync.dma_start(out=t, in_=dram[a:b, :])
        tiles.append(t)
    return tiles


def _body(nc, tc, dins, out_d):
    # Pool allocation order is the (LIFO) release order, reversed.  Base pools
    # live to the end; big transients nest inside phase windows.
    const = tc.alloc_tile_pool(name="const", bufs=1)
    ppool = tc.alloc_tile_pool(name="psum", bufs=2, space="PSUM")
    sepool = tc.alloc_tile_pool(name="se", bufs=1)        # s1/e1 (+ early ones rows)
    ht0pool = tc.alloc_tile_pool(name="ht0", bufs=1)      # f/br; reused as blend out
    trans = tc.alloc_tile_pool(name="trans", bufs=1)      # released end of phase E
    ht1pool = tc.alloc_tile_pool(name="ht1", bufs=1)      # f/b/br; released end of E
    xtpool = tc.alloc_tile_pool(name="xtilde", bufs=1)    # x~ slots shared by L0/L1
    ht0tmp = tc.alloc_tile_pool(name="ht0tmp", bufs=1)    # b/fr; released end of C

    ident = const.tile([128, 128], BF16)
    make_identity(nc, ident)
    c0sb = const.tile([128, 4], F32)
    nc.sync.dma_start(out=c0sb, in_=dins["c0f"])
    # Engine APs must start at a 32-aligned partition, so "ones" rows living at
    # odd partitions are written via SBUF->SBUF DMA from this partition-0 tile.
    ones_c = const.tile([1, BL, T + 1], BF16)
    nc.vector.memset(ones_c, 1.0)

    def init_ht(ht):
        # fresh-SBUF init in ONE DMA (DMA instructions only support one wait):
        # zeros + h0 at slot 0 + the ones rail for the bias rows.
        nc.sync.dma_start(out=ht, in_=dins["hti"])

    # All recurrence state tensors are allocated and initialized up front, on
    # fresh SBUF, so their init DMAs carry at most one sync wait each (the DMA
    # lowering only supports a single wait condition).
    ht0 = {}
    ht1 = {}
    ht0["f"] = ht0pool.tile([128, 4, BL, T + 1], BF16, name="ht0f", tag="ht0f")
    ht0["br"] = ht0pool.tile([128, 4, BL, T + 1], BF16, name="ht0br", tag="ht0br")
    ht0["b"] = ht0tmp.tile([128, 4, BL, T + 1], BF16, name="ht0b", tag="ht0b")
    ht0["fr"] = ht0tmp.tile([128, 4, BL, T + 1], BF16, name="ht0fr", tag="ht0fr")
    ht1["f"] = ht1pool.tile([128, 4, BL, T + 1], BF16, name="ht1f", tag="ht1f")
    ht1["b"] = ht1pool.tile([128, 4, BL, T + 1], BF16, name="ht1b", tag="ht1b")
    ht1["br"] = ht1pool.tile([128, 4, BL, T + 1], BF16, name="ht1br", tag="ht1br")
    for t_ in (ht0["f"], ht0["b"], ht1["f"], ht1["b"]):
        init_ht(t_)
    s1T = {}
    for nm in ("s", "e"):
        st = sepool.tile([128, 2, L], BF16, name=nm + "1T", tag=nm + "1T")
        nc.sync.dma_start(out=st[F - 128:F - 127, 1, :],
                          in_=ones_c.rearrange("p b t -> p (b t)")[:, 0:L])
        s1T[nm] = st

    def psum_tile():
        pz = ppool.tile([128, 4, GP], F32, name="pz", tag="pz")
        return pz

    # -------- phase A: layer-0 x_tilde (feature-major) --------
    xpool = tc.alloc_tile_pool(name="xt", bufs=1)
    xt_sb = _load_ktiles(nc, xpool, dins["xT"], KD, L, "xt")
    xtr_sb = _load_ktiles(nc, xpool, dins["xTr"], KD, L, "xtr")

    xt0 = {}
    for s, (wname, mov) in (("f", ("wx0f", xt_sb)), ("b", ("wx0b", xtr_sb))):
        wpool = tc.alloc_tile_pool(name="wx0" + s, bufs=1)
        wt = _load_ktiles(nc, wpool, dins[wname], KD, 3 * GP, "wx0" + s)
        store = xtpool.tile([128, NM, GP], BF16, name="xt0" + s, tag="xt" + s)
        for grp in range(3):
            pz = psum_tile()
            for mi in range(4):
                m = grp * 4 + mi
                for k in range(6):
                    nc.tensor.matmul(pz[:, mi, :], _r(wt[k][:, m * 128:(m + 1) * 128]),
                                     _r(mov[k]), start=(k == 0), stop=(k == 5))
            nc.scalar.copy(store[:, grp * 4:(grp + 1) * 4, :], pz)
        xt0[s] = store
        wpool.release()
    xpool.release()

    def lstm_jacobi_pair(streams):
        """Iterate both directions' LSTM fixed points together so the two
        streams' matmuls, activations and scans overlap across engines.
        streams = [(wh_tiles, xs, ht), ...]; ht is [128, 4, 2, 257],
        pre-initialized (slot 0 = h0, ones rail at [16, 3], zeros)."""
        for _ in range(K_ITERS):
            for si, (wh_tiles, xs, ht) in enumerate(streams):
                ss = str(si)
                I = trans.tile([128, 4, BL, T], F32, name="I" + ss, tag="I" + ss)
                Gt = trans.tile([128, 4, BL, T], F32, name="Gt" + ss, tag="Gt" + ss)
                O = trans.tile([128, 4, BL, T], F32, name="O" + ss, tag="O" + ss)
                Ct = trans.tile([128, 4, BL, T], F32, name="Ct" + ss, tag="Ct" + ss)
                for g, (dst, fn) in enumerate(((I, AF.Sigmoid), (Gt, AF.Tanh),
                                               (O, AF.Sigmoid))):
                    pz = psum_tile()
                    for mi in range(4):
                        m = g * 4 + mi
                        nc.tensor.matmul(pz[:, mi, :], _r(ident), _r(xs[:, m, :]),
                                         start=True, stop=False)
                        for k in range(4):
                            a, bnd = KH[k]
                            nc.tensor.matmul(pz[:, mi, :],
                                             _r(wh_tiles[k][:, m * 128:(m + 1) * 128]),
                                             _r(ht[0:bnd - a, k, :, 0:T]),
                                             start=False, stop=(k == 3))
                    nc.scalar.activation(dst, pz.rearrange("p m (b t) -> p m b t", b=BL), fn)
                nc.vector.tensor_mul(Gt, I, Gt)                     # b_t = i * g
                nc.vector.tensor_scalar(out=I, in0=I, scalar1=-1.0, scalar2=1.0,
                                        op0=mybir.AluOpType.mult, op1=mybir.AluOpType.add)
                for k in range(4):                                   # c scan per (chunk, b)
                    for b in range(BL):
                        nc.vector.tensor_tensor_scan(
                            out=Ct[:, k, b, :], data0=I[:, k, b, :], data1=Gt[:, k, b, :],
                            initial=c0sb[:, k:k + 1],
                            op0=mybir.AluOpType.mult, op1=mybir.AluOpType.add)
                nc.scalar.activation(Ct, Ct, AF.Tanh)
                nc.vector.tensor_mul(ht[:, 0:3, :, 1:T + 1], Ct[:, 0:3], O[:, 0:3])
                nc.vector.tensor_mul(ht[0:16, 3, :, 1:T + 1], Ct[0:16, 3], O[0:16, 3])

    # -------- phase B: layer-0 recurrences (both directions interleaved) -----
    whpool = tc.alloc_tile_pool(name="wh0", bufs=1)
    whf0 = _load_ktiles(nc, whpool, dins["wh0f"], KH, 3 * GP, "wh0f")
    whb0 = _load_ktiles(nc, whpool, dins["wh0b"], KH, 3 * GP, "wh0b")
    lstm_jacobi_pair([(whf0, xt0["f"], ht0["f"]), (whb0, xt0["b"], ht0["b"])])
    whpool.release()

    # reversed-time copies (the ones rail at [16, 3] copies over too)
    nc.vector.tensor_copy(ht0["fr"][:, :, :, 1:T + 1], ht0["f"][:, :, :, T:0:-1])
    nc.vector.tensor_copy(ht0["br"][:, :, :, 1:T + 1], ht0["b"][:, :, :, T:0:-1])

    # -------- phase C: layer-1 x_tilde --------
    xt1 = {}
    for s, (hfmov, hbmov) in (("f", (ht0["f"], ht0["br"])), ("b", (ht0["fr"], ht0["b"]))):
        wpool = tc.alloc_tile_pool(name="wx1" + s, bufs=1)
        wtf = _load_ktiles(nc, wpool, dins["wx1" + s + "f"], KH, 3 * GP, "wx1" + s + "f")
        wtb = _load_ktiles(nc, wpool, dins["wx1" + s + "b"], KH, 3 * GP, "wx1" + s + "b")
        store = xtpool.tile([128, NM, GP], BF16, name="xt1" + s, tag="xt" + s)
        pairs = [(wtf[k], hfmov, k) for k in range(4)] + [(wtb[k], hbmov, k) for k in range(4)]
        for grp in range(3):
            pz = psum_tile()
            for mi in range(4):
                m = grp * 4 + mi
                for pi, (wt, mov, k) in enumerate(pairs):
                    a, bnd = KH[k]
                    nc.tensor.matmul(pz[:, mi, :], _r(wt[:, m * 128:(m + 1) * 128]),
                                     _r(mov[0:bnd - a, k, :, 1:T + 1]),
                                     start=(pi == 0), stop=(pi == 7))
            nc.scalar.copy(store[:, grp * 4:(grp + 1) * 4, :], pz)
        xt1[s] = store
        wpool.release()
    ht0tmp.release()

    # -------- phase D: layer-1 recurrences (both directions interleaved) -----
    whpool = tc.alloc_tile_pool(name="wh1", bufs=1)
    whf1 = _load_ktiles(nc, whpool, dins["wh1f"], KH, 3 * GP, "wh1f")
    whb1 = _load_ktiles(nc, whpool, dins["wh1b"], KH, 3 * GP, "wh1b")
    lstm_jacobi_pair([(whf1, xt1["f"], ht1["f"]), (whb1, xt1["b"], ht1["b"])])
    whpool.release()
    nc.vector.tensor_copy(ht1["br"][:, :, :, 1:T + 1], ht1["b"][:, :, :, T:0:-1])
    xtpool.release()

    # -------- phase E: highway gate + blend (in place over ht0 f/br slots) ----
    hwpool = tc.alloc_tile_pool(name="hw", bufs=1)
    whf = _load_ktiles(nc, hwpool, dins["whwf"], KH, 2 * GP, "whwf")
    whb = _load_ktiles(nc, hwpool, dins["whwb"], KH, 2 * GP, "whwb")
    outT = {}
    pairs = [(whf[k], ht1["f"], k) for k in range(4)] + [(whb[k], ht1["br"], k) for k in range(4)]
    for half, (h1, h0) in (("f", (ht1["f"], ht0["f"])), ("b", (ht1["br"], ht0["br"]))):
        pz = psum_tile()
        for mi in range(4):
            m = (0 if half == "f" else 4) + mi
            for pi, (wt, mov, k) in enumerate(pairs):
                a, bnd = KH[k]
                nc.tensor.matmul(pz[:, mi, :], _r(wt[:, m * 128:(m + 1) * 128]),
                                 _r(mov[0:bnd - a, k, :, 1:T + 1]),
                                 start=(pi == 0), stop=(pi == 7))
        gate = trans.tile([128, 4, BL, T], F32, name="gate", tag="I0")
        nc.scalar.activation(gate, pz.rearrange("p m (b t) -> p m b t", b=BL), AF.Sigmoid)
        tmp = trans.tile([128, 4, BL, T], F32, name="tmpb", tag="Gt0")
        hsl = h0[:, :, :, 1:T + 1]
        nc.vector.tensor_sub(tmp, h1[:, :, :, 1:T + 1], hsl)
        nc.vector.tensor_mul(tmp, gate, tmp)
        # the final write skips partition 16 of chunk 3 so the ones rail from
        # the init image survives for the projection bias rows
        nc.vector.tensor_add(hsl[:, 0:3], hsl[:, 0:3], tmp[:, 0:3])
        nc.vector.tensor_add(hsl[0:16, 3], hsl[0:16, 3], tmp[0:16, 3])
        outT[half] = h0
    hwpool.release()
    ht1pool.release()
    trans.release()

    # -------- phase F: s/e projections --------
    for nm in ("s", "e"):
        wpool = tc.alloc_tile_pool(name="w" + nm, bufs=1)
        wf = _load_ktiles(nc, wpool, dins["w" + nm + "f"], KH, F, "w" + nm + "f")
        wb = _load_ktiles(nc, wpool, dins["w" + nm + "b"], KH, F, "w" + nm + "b")
        st = s1T[nm]
        prs = [(wf[k], outT["f"], k) for k in range(4)] + [(wb[k], outT["b"], k) for k in range(4)]
        pz = psum_tile()
        for mi, (ma, mb) in enumerate(((0, 128), (128, F))):
            for pi, (wt, mov, k) in enumerate(prs):
                a, bnd = KH[k]
                nc.tensor.matmul(pz[0:mb - ma, mi, :], _r(wt[:, ma:mb]),
                                 _r(mov[0:bnd - a, k, :, 1:T + 1]),
                                 start=(pi == 0), stop=(pi == 7))
        nc.scalar.copy(st[:, 0, :], pz[:, 0, :])
        nc.scalar.copy(st[0:F - 128, 1, :], pz[0:F - 128, 1, :])
        wpool.release()

    # -------- phase G: biaffine part 1: tmp[(c,j), (b,t)] --------
    biapool = tc.alloc_tile_pool(name="bia", bufs=1)
    upool = tc.alloc_tile_pool(name="u", bufs=1)
    KU = [(0, 128), (128, F + 1)]
    ut = _load_ktiles(nc, upool, dins["upk"], KU, C * 256, "upk")
    smov = [s1T["s"][:, 0, :], s1T["s"][0:F + 1 - 128, 1, :]]
    tmpT = biapool.tile([128, 16, GP], BF16, name="tmpT", tag="tmpT")
    for grp in range(4):
        pz = psum_tile()
        for mi in range(4):
            m = grp * 4 + mi
            for k in range(2):
                nc.tensor.matmul(pz[:, mi, :], _r(ut[k][:, m * 128:(m + 1) * 128]),
                                 _r(smov[k]), start=(k == 0), stop=(k == 1))
        nc.scalar.copy(tmpT[:, grp * 4:(grp + 1) * 4, :], pz)
    upool.release()

    # -------- phase H: biaffine part 2 + output assembly --------
    emov0 = s1T["e"][:, 0, :].rearrange("p (b t) -> p b t", b=BL)
    emov1 = s1T["e"][0:F + 1 - 128, 1, :].rearrange("p (b t) -> p b t", b=BL)
    ssbpool = tc.alloc_tile_pool(name="osb", bufs=2)
    for b in range(BL):
        for xt_i in range(2):
            osb = ssbpool.tile([128, T, C], F32, name="osb", tag="osb")
            pz = psum_tile()
            for c in range(C):
                xsl = slice(b * T + xt_i * 128, b * T + xt_i * 128 + 128)
                po = pz[:, c // 2, (c % 2) * T:(c % 2) * T + T]
                nc.tensor.matmul(po, _r(tmpT[:, 2 * c, xsl]), _r(emov0[:, b, :]),
                                 start=True, stop=False)
                nc.tensor.matmul(po, _r(tmpT[0:F + 1 - 128, 2 * c + 1, xsl]),
                                 _r(emov1[:, b, :]), start=False, stop=True)
            for c in range(C):
                nc.vector.tensor_copy(osb[:, :, c], pz[:, c // 2, (c % 2) * T:(c % 2) * T + T])
            nc.sync.dma_start(out=out_d[b, xt_i * 128:(xt_i + 1) * 128, :, :], in_=osb)
    ssbpool.release()
    biapool.release()
    ht0pool.release()
    sepool.release()
    ppool.release()
    const.release()


# ------------------------------------------------------------------ entry point

TRACE = False          # set True (from test harnesses) to capture an NTFF profile
LAST_RESULT = None     # BassKernelResults of the most recent run


def kernel(**inputs) -> np.ndarray:
    global LAST_RESULT
    if "nc" not in _CACHE:
        _CACHE["nc"] = _build_program()
    nc = _CACHE["nc"]
    in_maps = _pack_inputs(inputs)
    try:
        res = run_bass_kernel_spmd(nc, in_maps, core_ids=list(range(NCORES)),
                                   trace=TRACE)
    except ModuleNotFoundError:
        # no NTFF profile hook in this container; run without tracing
        res = run_bass_kernel_spmd(nc, in_maps, core_ids=list(range(NCORES)))
    LAST_RESULT = res
    out = np.concatenate([res.results[c]["out"] for c in range(NCORES)], axis=0)
    return np.ascontiguousarray(out.astype(np.float32))


if __name__ == "__main__":
    rng = np.random.default_rng(0)
    raise SystemExit("use test.py")
